# revision 1
# baseline (speedup 1.0000x reference)
"""Adaptive embedding (4-bucket) lookup + projection on 8 TRN2 NeuronCores.

Strategy: pure data-parallel over the 16384 tokens (no collectives).
  Host: bucket every token by its embedding table, deduplicate each table to
        the rows actually referenced (<= n_tokens distinct rows, so gather
        indices always fit int16), sort each bucket's tokens by row for HBM
        locality, and deal them evenly across the 8 cores so every core runs
        an identical-shape program.  Tables are pre-cast to bf16 with rows
        padded to a multiple of 128 elements; projections are pre-transposed,
        pre-scaled by sqrt(D) and zero-padded to match.
  Core: one dma_gather(transpose=True) per table pulls that bucket's
        embedding rows from HBM directly into d-on-partitions (matmul lhsT)
        layout; accumulating matmuls against the resident projT produce
        [128 tokens, 1024] in PSUM; DVE/ACT alternate evacuating to bf16 in
        SBUF; plain DMA stores the rows.
  Host: rows are scattered back to original token order and upcast to f32.
"""

import os
import sys

import numpy as np

for _p in ("/opt/trn_rl_repo",):
    if _p not in sys.path:
        sys.path.insert(0, _p)

import ml_dtypes

BF16 = ml_dtypes.bfloat16

N_TOKEN = 267735
CUTS = (0, 20000, 40000, 200000, N_TOKEN)
D_TBL = (1024, 256, 64, 16)
D_PAD = (1024, 256, 128, 128)
D_OUT = 1024
EMB_SCALE = float(D_OUT) ** 0.5
N_CORES = 8
P = 128

_PROGRAM_CACHE = {}
LAST_RESULTS = None  # BassKernelResults of the most recent run (for profiling)


def _build_program(active, slot_counts, out_counts, tbl_rows):
    """Build + compile the per-core Bass program.

    active: tuple of table ids with nonzero token count
    slot_counts / out_counts: per active table — gather slots (mult of 128)
        and output row count (identical on every core)
    tbl_rows: rows of each deduplicated bf16 table
    """
    import concourse.bacc as bacc
    import concourse.mybir as mybir
    import concourse.tile as tile

    dt = mybir.dt
    nc = bacc.Bacc("TRN2", target_bir_lowering=False, debug=False,
                   num_swdge_queues=4)

    embs = {
        t: nc.dram_tensor(f"embt{t}", [tbl_rows[t], D_PAD[t]], dt.bfloat16,
                          kind="ExternalInput")
        for t in active
    }
    projs = {
        t: nc.dram_tensor(f"projt{t}", [D_PAD[t], D_OUT], dt.bfloat16,
                          kind="ExternalInput")
        for t in active
    }
    total_slots = sum(slot_counts[t] for t in active)
    idx = nc.dram_tensor("idx", [P, total_slots // 16], dt.int16,
                         kind="ExternalInput")
    R = sum(out_counts[t] for t in active)
    outb = nc.dram_tensor("outb", [R, D_OUT], dt.bfloat16, kind="ExternalOutput")

    from concourse.library_config import mlp

    with tile.TileContext(nc) as tc:
        with (
            tc.tile_pool(name="const", bufs=1) as const_pool,
            tc.tile_pool(name="gath", bufs=1) as gath_pool,
            tc.tile_pool(name="evac", bufs=1) as evac_pool,
            tc.tile_pool(name="psum", bufs=8, space="PSUM") as psum_pool,
        ):
            # the Q7 mlp library (dma_gather) takes ~10us to land — start the
            # load as early as possible
            nc.gpsimd.load_library(mlp)

            # all token-index tiles in one small DMA, first in the queue
            idx_sb = const_pool.tile([P, total_slots // 16], dt.int16, tag="idx")
            nc.sync.dma_start(idx_sb[:], idx[:])

            # gathers: rows land transposed, [128, K, C] = emb^T K-tiles.
            # The Q7 gather kernel's index scratch caps num_idxs (~1K crashes
            # on HW) — split big gathers into <=MAX_GATHER column slices, and
            # spread pieces across the 4 SWDGE queues (distinct Q7 core
            # pairs) so their descriptor generation runs concurrently.
            MAX_GATHER = 768
            pieces = []  # (table, tile, col0, size, idx_off)
            gath_sb = {}
            off = 0
            for t in active:
                K = D_PAD[t] // P
                C = slot_counts[t]
                gt = gath_pool.tile([P, K, C], dt.bfloat16, tag=f"g{t}")
                n_piece = -(-C // MAX_GATHER)
                piece = -(-(C // P) // n_piece) * P
                assert n_piece == 1 or K == 1
                for c0 in range(0, C, piece):
                    cs = min(piece, C - c0)
                    pieces.append((t, gt, c0, cs, off + c0, n_piece > 1))
                gath_sb[t] = gt
                off += C
            # schedule: table 0 first (its matmuls gate the PE start; the
            # first-dispatched gather begins ~2us before the rest), then big
            # pieces, round-robin over the 4 queues.
            # NOTE: overflow gathers (beyond one per queue) must cycle back
            # to queue 0 — a second gather issued on queue 3 while others
            # are in flight corrupts lanes 4/6/7 of concurrent gathers
            # (HW-reproduced; see probe5 experiments).
            pieces.sort(key=lambda p: (p[0] != 0, -p[3]))
            g0_inst = None
            for i, (t, gt, c0, cs, ioff, sliced) in enumerate(pieces):
                q = i % 4
                gi = nc.gpsimd.dma_gather(
                    gt[:, :, c0:c0 + cs] if sliced else gt[:],
                    embs[t][:, :],
                    idx_sb[:, ioff // 16:(ioff + cs) // 16],
                    cs,
                    cs,
                    D_PAD[t],
                    transpose=True,
                    queue_num=q,
                )
                if g0_inst is None:
                    g0_inst = gi

            # resident projections: [Dp, 1024] -> [128, K, 1024].
            # Split each into per-K-tile DMAs so the first matmuls only wait
            # for the K-tiles they read.  (Delaying these behind the first
            # gather was tried to unclog the library-image load — it made the
            # mean worse; the per-core library-load variance is not projT
            # traffic.)
            proj_sb = {}
            for t in active:
                K = D_PAD[t] // P
                pt = const_pool.tile([P, K, D_OUT], dt.bfloat16, tag=f"proj{t}")
                src = projs[t][:, :].rearrange("(k p) n -> p k n", p=P)
                for k in range(K):
                    nc.sync.dma_start(pt[:, k, :], src[:, k, :])
                proj_sb[t] = pt

            # per 128-token chunk: accumulate over K into PSUM; as soon as
            # each 512-wide bank's chain completes, evacuate that half on
            # DVE / ACT (one engine per half, in parallel); store each
            # table with 1-2 big DMAs from a per-table staging tile
            row0 = 0
            for t in active:
                K = D_PAD[t] // P
                n_c = -(-out_counts[t] // P)
                ev = evac_pool.tile([P, n_c, D_OUT], dt.bfloat16, tag=f"ev{t}")
                for c in range(n_c):
                    for n in range(2):
                        ps = psum_pool.tile([P, 512], dt.float32, tag="ps")
                        for kt in range(K):
                            nc.tensor.matmul(
                                ps[:],
                                gath_sb[t][:, kt, c * P:(c + 1) * P],
                                proj_sb[t][:, kt, n * 512:(n + 1) * 512],
                                start=(kt == 0),
                                stop=(kt == K - 1),
                            )
                        half = ev[:, c, n * 512:(n + 1) * 512]
                        if n == 0:
                            nc.vector.tensor_copy(half, ps[:])
                        else:
                            nc.scalar.copy(half, ps[:])
                fc, rem = divmod(out_counts[t], P)
                # store in 3-chunk groups so transfers start mid-compute and
                # the final (tail) store is small
                groups = [(i, min(i + 3, fc)) for i in range(0, max(fc, 1), 3)]
                for ca, cb in groups:
                    if cb > ca:
                        nc.sync.dma_start(
                            outb[row0 + ca * P:row0 + cb * P, :]
                            .rearrange("(c p) n -> p c n", p=P),
                            ev[:, ca:cb, :],
                        )
                if rem:
                    nc.sync.dma_start(
                        outb[row0 + fc * P: row0 + fc * P + rem, :],
                        ev[:rem, fc, :],
                    )
                row0 += out_counts[t]

    nc.finalize()
    return nc


def _host_prep(inp):
    """Bucket tokens by table; dedup rows; sort by row; per-core counts."""
    flat = np.asarray(inp).reshape(-1).astype(np.int64)

    tbl = np.searchsorted(np.asarray(CUTS[1:]), flat, side="right")
    local = flat - np.asarray(CUTS)[tbl]

    positions = {}
    lidx = {}
    uniq = {}
    for t in range(4):
        pos = np.nonzero(tbl == t)[0]
        if not pos.size:
            continue
        rows = local[pos]
        u, inv = np.unique(rows, return_inverse=True)
        order = np.argsort(inv, kind="stable")   # sort tokens by table row
        positions[t] = pos[order]
        lidx[t] = inv[order].astype(np.int16)
        uniq[t] = u

    active = tuple(sorted(positions.keys()))
    out_counts = {}
    slot_counts = {}
    for t in active:
        n = len(positions[t])
        cg = -(-n // N_CORES)           # ceil(n / 8): rows per core
        out_counts[t] = cg
        slot_counts[t] = max(P, -(-cg // P) * P)
    return flat, active, positions, lidx, uniq, out_counts, slot_counts


def _idx_tensor(active, lidx, slot_counts, core):
    """Combined int16 [128, total_slots/16] tile for one core.

    Slot j of a group at [j%16, j//16] within the group's column window;
    pads read row 0.  HW's dma_gather on SWDGE queue q reads the indices
    from partitions 32q+16 .. 32q+31 while CoreSim reads 0-15 — write all
    five ranges so any queue assignment (and the sim) sees them.
    """
    total = sum(slot_counts[t] for t in active)
    arr = np.zeros((P, total // 16), np.int16)
    off = 0
    for t in active:
        li = lidx[t][core::N_CORES]
        j = np.arange(len(li))
        for base in (0, 16, 48, 80, 112):
            arr[base + j % 16, off // 16 + j // 16] = li
        off += slot_counts[t]
    return arr


def _prep_compact_tables(active, uniq, raw_tables, raw_projs):
    tables = {}
    projTs = {}
    for t in active:
        emb = raw_tables[t]
        sel = np.asarray(emb, dtype=np.float32)[uniq[t]]
        tb = np.zeros((len(uniq[t]), D_PAD[t]), BF16)
        tb[:, :emb.shape[1]] = sel.astype(BF16)
        tables[t] = tb
        proj = raw_projs[t]
        pt = np.zeros((D_PAD[t], D_OUT), np.float32)
        pt[:proj.shape[1], :] = (np.asarray(proj, np.float32) * EMB_SCALE).T
        projTs[t] = pt.astype(BF16)
    return tables, projTs


def kernel(inp, emb0, emb1, emb2, emb3, proj0, proj1, proj2, proj3):
    global LAST_RESULTS
    from concourse.bass_utils import run_bass_kernel_spmd

    flat, active, positions, lidx, uniq, out_counts, slot_counts = \
        _host_prep(inp)
    T = flat.shape[0]

    tables, projTs = _prep_compact_tables(
        active, uniq, (emb0, emb1, emb2, emb3), (proj0, proj1, proj2, proj3))
    tbl_rows = {t: tables[t].shape[0] for t in active}

    key = (active, tuple(slot_counts[t] for t in active),
           tuple(out_counts[t] for t in active),
           tuple(tbl_rows[t] for t in active))
    nc = _PROGRAM_CACHE.get(key)
    if nc is None:
        nc = _build_program(active, slot_counts, out_counts, tbl_rows)
        _PROGRAM_CACHE[key] = nc

    in_maps = []
    for k in range(N_CORES):
        m = {}
        for t in active:
            m[f"embt{t}"] = tables[t]
            m[f"projt{t}"] = projTs[t]
        m["idx"] = _idx_tensor(active, lidx, slot_counts, k)
        in_maps.append(m)

    trace = bool(os.environ.get("KERNEL_TRACE"))
    res = run_bass_kernel_spmd(nc, in_maps, core_ids=list(range(N_CORES)),
                               trace=trace)
    LAST_RESULTS = res

    out = np.empty((T, D_OUT), np.float32)
    bases = {}
    r0 = 0
    for t in active:
        bases[t] = r0
        r0 += out_counts[t]
    for k in range(N_CORES):
        ob = np.asarray(res.results[k]["outb"])
        for t in active:
            pos = positions[t][k::N_CORES]
            if pos.size:
                out[pos] = ob[bases[t]:bases[t] + len(pos)].astype(np.float32)

    return out.reshape(*np.asarray(inp).shape, D_OUT)



# revision 10
# speedup vs baseline: 1.0752x; 1.0752x over previous
"""Adaptive embedding (4-bucket) lookup + projection on 8 TRN2 NeuronCores.

Strategy: pure data-parallel over the 16384 tokens (no collectives).
  Host: bucket every token by its embedding table, deduplicate each table to
        the rows actually referenced, sort each bucket's tokens by row for HBM
        locality, and deal them evenly across the 8 cores so every core runs
        an identical-shape program.  Tables are pre-cast to bf16 (unpadded);
        projections are pre-transposed, pre-scaled by sqrt(D).
  Core: stock per-chunk indirect_dma_start calls (built-in Q7 firmware — no
        mlp library, whose ~11us IRAM load dominated the old critical path)
        gather each 128-token chunk token-on-partition; the PE flips blocks
        to d-on-partitions via identity-matmul transposes (4 blocks packed
        per PSUM bank, one evac per group); projection matmuls run in two
        orientations:
          - t0/t1 (K=8/2 k-tiles): token-chunk stationary [128k x 128tok]
            x projT[128k, 512] accumulating over K into [tok, dout] PSUM
            (each LDWEIGHTS reused across the two 512-halves),
          - t2/t3 (K=64/16 partitions): proj-stationary [K x 128dout]
            x eT[K, tok] giving [dout, tok] PSUM with exact token columns
            (no chunk padding) and only 8 LDWEIGHTS per table.
        DVE/ACT alternate evacuating PSUM to bf16; stores stream per slice.
        Table order 0,2,3,1 so the final (tail) store is the smallest.
  Host: rows are scattered back to original token order and upcast to f32
        (t2/t3 arrive dout-major and are transposed on host).
"""

import os
import sys

import numpy as np

for _p in ("/opt/trn_rl_repo",):
    if _p not in sys.path:
        sys.path.insert(0, _p)

import ml_dtypes

BF16 = ml_dtypes.bfloat16

N_TOKEN = 267735
CUTS = (0, 20000, 40000, 200000, N_TOKEN)
D_TBL = (1024, 256, 64, 16)
D_OUT = 1024
EMB_SCALE = float(D_OUT) ** 0.5
N_CORES = 8
P = 128
TBL_ORDER = (0, 2, 3, 1)   # compute/store order: smallest store last

_PROGRAM_CACHE = {}
LAST_RESULTS = None  # BassKernelResults of the most recent run (for profiling)


def _build_program(active, out_counts, tbl_rows):
    """Build + compile the per-core Bass program.

    active: table ids with nonzero token count, in processing order
    out_counts: per active table — token rows per core (identical on every
        core; real rows on the last cores may be fewer, host slices)
    tbl_rows: rows of each deduplicated bf16 table
    """
    import concourse.bacc as bacc
    import concourse.bass as bass
    import concourse.mybir as mybir
    import concourse.tile as tile

    dt = mybir.dt
    nc = bacc.Bacc("TRN2", target_bir_lowering=False, debug=False,
                   num_swdge_queues=1)

    chunks = {t: -(-out_counts[t] // P) for t in active}

    embs = {
        t: nc.dram_tensor(f"embt{t}", [tbl_rows[t], D_TBL[t]], dt.bfloat16,
                          kind="ExternalInput")
        for t in active
    }
    projs = {
        t: nc.dram_tensor(f"projt{t}", [D_TBL[t], D_OUT], dt.bfloat16,
                          kind="ExternalInput")
        for t in active
    }
    total_chunks = sum(chunks[t] for t in active)
    idx = nc.dram_tensor("idx", [P, total_chunks], dt.int32,
                         kind="ExternalInput")
    ident = nc.dram_tensor("ident", [P, P], dt.bfloat16, kind="ExternalInput")
    outs = {}
    for t in active:
        if D_TBL[t] >= P:
            outs[t] = nc.dram_tensor(f"outb{t}", [out_counts[t], D_OUT],
                                     dt.bfloat16, kind="ExternalOutput")
        else:
            outs[t] = nc.dram_tensor(f"outb{t}", [D_OUT, out_counts[t]],
                                     dt.bfloat16, kind="ExternalOutput")

    with tile.TileContext(nc) as tc:
        with (
            tc.tile_pool(name="const", bufs=1) as const_pool,
            tc.tile_pool(name="gath", bufs=1) as gath_pool,
            tc.tile_pool(name="evac", bufs=1) as evac_pool,
            tc.tile_pool(name="psum", bufs=8, space="PSUM") as psum_pool,
        ):
            # token-index + identity tiles: small DMAs, first in the queue
            idx_sb = const_pool.tile([P, total_chunks], dt.int32, tag="idx")
            nc.sync.dma_start(idx_sb[:], idx[:])
            id_sb = const_pool.tile([P, P], dt.bfloat16, tag="id")
            nc.sync.dma_start(id_sb[:], ident[:])

            # stock indirect gathers (built-in Q7 firmware, no library):
            # one [128,1]-offset call per 128-token chunk, rows land
            # token-on-partition [128, chunk, d].
            gath_sb = {}
            off = 0
            for t in active:
                d, c = D_TBL[t], chunks[t]
                gt = gath_pool.tile([P, c, d], dt.bfloat16, tag=f"g{t}",
                                    name=f"g{t}")
                for cc in range(c):
                    nc.gpsimd.indirect_dma_start(
                        out=gt[:, cc, :],
                        out_offset=None,
                        in_=embs[t][:, :],
                        in_offset=bass.IndirectOffsetOnAxis(
                            ap=idx_sb[:, off + cc:off + cc + 1], axis=0),
                    )
                gath_sb[t] = gt
                off += c

            # resident projections, one DMA per 128-row k-tile so the first
            # matmuls only wait for the k-tiles they read.  t2/t3 projT have
            # K=64/16 real rows living in partitions 0..K.
            proj_sb = {}
            for t in active:
                d = D_TBL[t]
                K = max(1, d // P)
                pt = const_pool.tile([P, K, D_OUT], dt.bfloat16, tag=f"p{t}",
                                     name=f"p{t}")
                if d >= P:
                    src = projs[t][:, :].rearrange("(k p) n -> p k n", p=P)
                    for k in range(K):
                        nc.sync.dma_start(pt[:, k, :], src[:, k, :])
                else:
                    nc.sync.dma_start(pt[0:d, 0, :], projs[t][:, :])
                proj_sb[t] = pt

            evac_flip = [0]

            def evac(dst, ps):
                if evac_flip[0] % 2 == 0:
                    nc.vector.tensor_copy(dst, ps)
                else:
                    nc.scalar.copy(dst, ps)
                evac_flip[0] += 1

            for t in active:
                d, c, n_rows = D_TBL[t], chunks[t], out_counts[t]
                K = max(1, d // P)
                dp = min(d, P)

                # --- PE transpose to d-on-partitions, 4 blocks per PSUM
                # bank, one evac per group.  eT layout:
                #   d>=P: [128, c, K, 128] (block b = (cc, k) = divmod(b, K))
                #   d< P: [dp, c*128]      (block b = chunk b)
                if d >= P:
                    et = evac_pool.tile([P, c * K, P], dt.bfloat16,
                                        tag=f"e{t}", name=f"e{t}")
                else:
                    et = evac_pool.tile([P, c * P], dt.bfloat16,
                                        tag=f"e{t}", name=f"e{t}")
                n_blk = c * K
                for b0 in range(0, n_blk, 4):
                    nb = min(4, n_blk - b0)
                    ps = psum_pool.tile([P, 512], dt.bfloat16, tag="ps",
                                        name=f"tp{t}_{b0}")
                    for i in range(nb):
                        b = b0 + i
                        cc, k = divmod(b, K)
                        src = (gath_sb[t][:, cc, k * P:(k + 1) * P]
                               if d >= P else gath_sb[t][:, cc, :])
                        nc.tensor.transpose(
                            ps[0:dp, i * P:(i + 1) * P], src, id_sb[:])
                    if d >= P:
                        dst = et[:, b0:b0 + nb, :]
                    else:
                        dst = et[0:dp, b0 * P:(b0 + nb) * P]
                    evac(dst, ps[0:dp, 0:nb * P])

                # --- projection matmuls + evac + stores
                if d >= P:
                    # orientation A: lhsT = eT chunk [128k, 128tok]
                    # stationary (reused across the two 512-halves),
                    # rhs = projT k-tile.
                    ev = evac_pool.tile([P, c, D_OUT], dt.bfloat16,
                                        tag=f"ev{t}", name=f"ev{t}")
                    for cc in range(c):
                        ps = [psum_pool.tile([P, 512], dt.float32, tag="ps",
                                             name=f"ps{t}_{cc}_{n}")
                              for n in range(2)]
                        for kt in range(K):
                            for n in range(2):
                                nc.tensor.matmul(
                                    ps[n][:],
                                    et[:, cc * K + kt, :],
                                    proj_sb[t][:, kt, n * 512:(n + 1) * 512],
                                    start=(kt == 0),
                                    stop=(kt == K - 1),
                                )
                        for n in range(2):
                            evac(ev[:, cc, n * 512:(n + 1) * 512], ps[n][:])
                    fc, rem = divmod(n_rows, P)
                    for cc in range(fc):
                        nc.sync.dma_start(
                            outs[t][cc * P:(cc + 1) * P, :], ev[:, cc, :])
                    if rem:
                        nc.sync.dma_start(
                            outs[t][fc * P:fc * P + rem, :],
                            ev[0:rem, fc, :])
                else:
                    # orientation B: lhsT = projT [d, 128dout] stationary
                    # (8 LDW total), rhs = eT [d, tok] with exact token
                    # columns; PSUM holds [128dout, <=512tok].
                    blocks = []
                    b0 = 0
                    while b0 < n_rows:
                        blocks.append((b0, min(b0 + 512, n_rows)))
                        b0 += 512
                    ev = evac_pool.tile([P, 8, n_rows], dt.bfloat16,
                                        tag=f"ev{t}", name=f"ev{t}")
                    for s in range(8):
                        for (c0, c1) in blocks:
                            ps = psum_pool.tile([P, 512], dt.float32,
                                                tag="ps",
                                                name=f"ps{t}_{s}_{c0}")
                            nc.tensor.matmul(
                                ps[:, 0:c1 - c0],
                                proj_sb[t][0:d, 0, s * P:(s + 1) * P],
                                et[0:d, c0:c1],
                                start=True,
                                stop=True,
                            )
                            evac(ev[:, s, c0:c1], ps[:, 0:c1 - c0])
                        nc.sync.dma_start(
                            outs[t][s * P:(s + 1) * P, :], ev[:, s, :])

    nc.finalize()
    return nc


def _host_prep(inp):
    """Bucket tokens by table; dedup rows; sort by row; per-core counts."""
    flat = np.asarray(inp).reshape(-1).astype(np.int64)

    tbl = np.searchsorted(np.asarray(CUTS[1:]), flat, side="right")
    local = flat - np.asarray(CUTS)[tbl]

    positions = {}
    lidx = {}
    uniq = {}
    for t in range(4):
        pos = np.nonzero(tbl == t)[0]
        if not pos.size:
            continue
        rows = local[pos]
        u, inv = np.unique(rows, return_inverse=True)
        order = np.argsort(inv, kind="stable")   # sort tokens by table row
        positions[t] = pos[order]
        lidx[t] = inv[order].astype(np.int32)
        uniq[t] = u

    active = tuple(t for t in TBL_ORDER if t in positions)
    out_counts = {}
    for t in active:
        out_counts[t] = -(-len(positions[t]) // N_CORES)  # ceil: rows/core
    return flat, active, positions, lidx, uniq, out_counts


def _idx_tensor(active, lidx, out_counts, core):
    """Combined int32 [128, total_chunks] index tile for one core.

    Token j of table t sits at [j % 128, chunk_base_t + j // 128]; pad
    slots read row 0.
    """
    total = sum(-(-out_counts[t] // P) for t in active)
    arr = np.zeros((P, total), np.int32)
    off = 0
    for t in active:
        li = lidx[t][core::N_CORES]
        j = np.arange(len(li))
        arr[j % P, off + j // P] = li
        off += -(-out_counts[t] // P)
    return arr


def _prep_compact_tables(active, uniq, raw_tables, raw_projs):
    tables = {}
    projTs = {}
    for t in active:
        emb = np.asarray(raw_tables[t], dtype=np.float32)
        tables[t] = emb[uniq[t]].astype(BF16)
        proj = np.asarray(raw_projs[t], np.float32)
        projTs[t] = np.ascontiguousarray((proj * EMB_SCALE).T).astype(BF16)
    return tables, projTs


def kernel(inp, emb0, emb1, emb2, emb3, proj0, proj1, proj2, proj3):
    global LAST_RESULTS
    from concourse.bass_utils import run_bass_kernel_spmd

    flat, active, positions, lidx, uniq, out_counts = _host_prep(inp)
    T = flat.shape[0]

    tables, projTs = _prep_compact_tables(
        active, uniq, (emb0, emb1, emb2, emb3), (proj0, proj1, proj2, proj3))
    tbl_rows = {t: tables[t].shape[0] for t in active}

    key = (active, tuple(out_counts[t] for t in active),
           tuple(tbl_rows[t] for t in active))
    nc = _PROGRAM_CACHE.get(key)
    if nc is None:
        nc = _build_program(active, out_counts, tbl_rows)
        _PROGRAM_CACHE[key] = nc

    ident = np.eye(P, dtype=np.float32).astype(BF16)
    in_maps = []
    for k in range(N_CORES):
        m = {}
        for t in active:
            m[f"embt{t}"] = tables[t]
            m[f"projt{t}"] = projTs[t]
        m["idx"] = _idx_tensor(active, lidx, out_counts, k)
        m["ident"] = ident
        in_maps.append(m)

    trace = bool(os.environ.get("KERNEL_TRACE"))
    res = run_bass_kernel_spmd(nc, in_maps, core_ids=list(range(N_CORES)),
                               trace=trace)
    LAST_RESULTS = res

    out = np.empty((T, D_OUT), np.float32)
    for k in range(N_CORES):
        for t in active:
            pos = positions[t][k::N_CORES]
            if not pos.size:
                continue
            ob = np.asarray(res.results[k][f"outb{t}"])
            if D_TBL[t] >= P:
                out[pos] = ob[:len(pos)].astype(np.float32)
            else:
                out[pos] = ob[:, :len(pos)].T.astype(np.float32)

    return out.reshape(*np.asarray(inp).shape, D_OUT)
